# revision 45
# baseline (speedup 1.0000x reference)
"""Causal single-head attention (B=4, S=2048, D=1024) on 8 trn2 NeuronCores.

Sharding: core = (batch b, parity h).  Each core owns the 1024 queries of
batch b in 256-row blocks {2t+h : t=0..3} (interleaved for causal load
balance), projects Q for its own rows, K for the full sequence, and V for
its own rows only (V halves are pair-wise all-gathered, hidden under the
K projection).  K stays replicated: a gathered K would put the collective
on the scores critical path, and concurrent DMA traffic halves the
collective's effective bandwidth (measured), so the 27us of duplicated K
matmuls are cheaper.

All inputs are pre-cast to bf16 on the host (the on-chip matmuls are bf16
anyway), halving input HBM traffic and removing the f32 staging casts.

On-chip dataflow (per core, SPMD-uniform):
  warmup: 24 throwaway matmuls on a zeroed tile while the first input
          loads land, so the PE HAM clock-gate is at 8/8 when real work
          starts.
  proj:  V_own[k,e] = xq^T Wv   (8 row-tiles; gathered pair-wise via one
                                 AllGather into V4[r][j])
         QT[e,q]    = Wq^T xq^T
         KT[e,k]    = Wk^T x^T   (full sequence, 4 key chunks of 512)
  attention per slot pair p (queries 512p..512p+511, nsh = 4(2p+1)):
         scoresT[k,q] = KT^T QT  over k-tiles 0..nsh-1 at N=512
                        (+ 4 extra k-tiles at N=256 for the later slot)
         expT = exp(scoresT/32) * mask    (multiplicative 0/1 causal mask)
         den[1,q] += ones^T expT          (matmul; [q]-major via DRAM
                                           roundtrip -> rinv[q,1])
         out[q,e]  = sum_k expT[k,q]^T V[k,e]   <- PV with expT stationary,
                     accumulated per 128-query tile directly in [q,e]
                     orientation (no PE transposes), scaled by rinv on DVE
                     and stored bf16.
"""

import os
import sys
from contextlib import ExitStack

import numpy as np
import ml_dtypes

import concourse.bass as bass
import concourse.mybir as mybir
import concourse.tile as tile
from concourse import bacc
from concourse import bass_utils

B, S, D = 4, 2048, 1024
P = 128
QB = 256          # queries per slot
NSLOT = 4         # slots per core
NQ = QB * NSLOT   # queries per core
NCORES = 8
F32 = mybir.dt.float32
BF16 = mybir.dt.bfloat16
SCALE = 1.0 / 32.0  # 1/sqrt(D)


def _build_kernel():
    nc = bacc.Bacc("TRN2", target_bir_lowering=False, debug=False,
                   num_devices=NCORES)

    xtd = nc.dram_tensor("xtd", [2, P, 8, 512], BF16, kind="ExternalInput").ap()
    xkd = nc.dram_tensor("xkd", [P, 8, 512], BF16, kind="ExternalInput").ap()
    xqd = nc.dram_tensor("xqd", [2, P, 8, 512], BF16, kind="ExternalInput").ap()
    wqd = nc.dram_tensor("wqd", [P, 8, D], BF16, kind="ExternalInput").ap()
    wkd = nc.dram_tensor("wkd", [P, 8, D], BF16, kind="ExternalInput").ap()
    wvd = nc.dram_tensor("wvd", [2, P, 8, 512], BF16, kind="ExternalInput").ap()
    maskT = nc.dram_tensor("maskT", [P, 2, P], BF16, kind="ExternalInput").ap()
    out = nc.dram_tensor("out", [NQ, D], BF16, kind="ExternalOutput").ap()
    # cores 2b (h=0) and 2b+1 (h=1) of batch b exchange V halves
    GROUPS = [[0, 1], [2, 3], [4, 5], [6, 7]]

    with tile.TileContext(nc) as tc, ExitStack() as ctx:
        const = ctx.enter_context(tc.tile_pool(name="const", bufs=1))
        persist = ctx.enter_context(tc.tile_pool(name="persist", bufs=1))

        ones = const.tile([P, 1], BF16)
        nc.gpsimd.memset(ones[:], 1.0)
        scratch = const.tile([P, 512], BF16)
        nc.gpsimd.memset(scratch[:], 0.0)
        mask_sb = const.tile([P, 2, P], BF16)

        QT = persist.tile([P, 8, NQ], BF16)      # [e_in_tile, e_tile, q]
        KT = persist.tile([P, 8, S], BF16)       # [e_in_tile, e_tile, k]
        V4 = persist.tile([P, 2, 8, D], BF16)    # [k_in_tile, parity, j, e]
        denT = persist.tile([P, 2 * NSLOT], F32)
        rinv = persist.tile([P, 2 * NSLOT], F32)

        # ---------------- projection phase ----------------
        # V-own first so the pair AllGather launches as early as possible;
        # its latency hides under the Q and K projections.
        with tc.tile_pool(name="wsb", bufs=1) as wsb_pool, \
             tc.tile_pool(name="xtp", bufs=2) as xt_pool, \
             tc.tile_pool(name="xqp", bufs=1) as xq_pool, \
             tc.tile_pool(name="vhp", bufs=1) as vh_pool, \
             tc.tile_pool(name="ccdram", bufs=1, space="DRAM") as ccdram, \
             tc.tile_pool(name="pproj", bufs=4, space="PSUM") as pproj:

            wq_sb = wsb_pool.tile([P, 8, D], BF16, tag="wq")
            wk_sb = wsb_pool.tile([P, 8, D], BF16, tag="wk")
            wv_sb = wsb_pool.tile([P, 2, 8, 512], BF16, tag="wv")
            xq_sb = xq_pool.tile([P, 2, 8, 512], BF16, tag="xq")
            xk_sb = xq_pool.tile([P, 8, 512], BF16, tag="xk")
            vh = vh_pool.tile([P, 8, D], BF16, tag="vh")
            kth = vh_pool.tile([P, 8, 512], BF16, tag="kth")

            # input loads, issued in consumption order (sync/HWDGE queue).
            # The first wv/xq halves are split by d-tile so the very first
            # matmul group can start after ~1MB instead of ~2MB.
            nc.sync.dma_start(wv_sb[:, 0, 0:4, :], wvd[0][:, 0:4, :])
            nc.sync.dma_start(xq_sb[:, 0, 0:4, :], xqd[0][:, 0:4, :])
            nc.sync.dma_start(wv_sb[:, 0, 4:8, :], wvd[0][:, 4:8, :])
            nc.sync.dma_start(xq_sb[:, 0, 4:8, :], xqd[0][:, 4:8, :])
            nc.sync.dma_start(xq_sb[:, 1], xqd[1])
            nc.sync.dma_start(wv_sb[:, 1], wvd[1])
            nc.sync.dma_start(wk_sb[:], wkd[:])
            xt_tiles = []
            for c in range(2):
                xt = xt_pool.tile([P, 8, 512], BF16, tag="xt")
                nc.sync.dma_start(xt[:], xtd[c])
                xt_tiles.append(xt)
            nc.sync.dma_start(xk_sb[:], xkd[:])
            nc.sync.dma_start(wq_sb[:], wqd[:])
            nc.sync.dma_start(mask_sb[:], maskT[:])

            # PE warmup on zeroed data while the first loads are in flight
            wps = pproj.tile([P, 512], F32, tag="pp")
            for i in range(16):
                nc.tensor.matmul(wps[0:1, :], scratch[:, 0:1], scratch[:],
                                 start=(i == 0), stop=(i == 15))

            # V_own[k,e]: stationary xq row-tile, moving wv e-half.
            # The first (eh=0, hh=0) pass runs d-tile-half-major so its
            # 4 psum groups can start on the first half-loads.
            ps4 = [pproj.tile([P, 512], F32, tag="pp", name=f"ps4_{i}")
                   for i in range(4)]
            for dh in range(2):
                for jj in range(4):
                    for dt in range(4 * dh, 4 * dh + 4):
                        nc.tensor.matmul(
                            ps4[jj][:],
                            xq_sb[:, 0, dt, P * jj:P * (jj + 1)],
                            wv_sb[:, 0, dt, :],
                            start=(dt == 0), stop=(dt == 7))
            for jj in range(4):
                nc.scalar.copy(vh[:, jj, 0:512], ps4[jj][:])

            def emit_v(eh, hh):
                for jj in range(4):
                    j = 4 * hh + jj
                    ps = pproj.tile([P, 512], F32, tag="pp")
                    for dt in range(8):
                        nc.tensor.matmul(
                            ps[:],
                            xq_sb[:, hh, dt, P * jj:P * (jj + 1)],
                            wv_sb[:, eh, dt, :],
                            start=(dt == 0), stop=(dt == 7))
                    nc.scalar.copy(vh[:, j, 512 * eh:512 * (eh + 1)], ps[:])

            emit_v(0, 1)
            emit_v(1, 0)
            emit_v(1, 1)

            # pair all-gather of V halves (runs on TOPSP/SDMA, overlapped);
            # the V4 unpacks go on the sync queue so they never delay the
            # second collective's trigger on the gpsimd queue
            ccv_in = ccdram.tile([P, 8 * D], BF16, tag="ccv_in")
            ccv_out = ccdram.tile([2, P, 8 * D], BF16, tag="ccv_out")
            nc.gpsimd.dma_start(ccv_in[:],
                                vh[:].rearrange("p a b -> p (a b)"))
            nc.gpsimd.collective_compute(
                "AllGather", mybir.AluOpType.bypass,
                replica_groups=GROUPS,
                ins=[ccv_in[:]], outs=[ccv_out[:]])
            # piece 0 unpacks on the sync queue; piece 1 is emitted on the
            # gpsimd queue after the K-AllGather's trigger (below), so the
            # two unpacks run on different DMA rings in parallel and PV of
            # window 0 unblocks sooner
            nc.sync.dma_start(
                V4[:, 0].rearrange("p a b -> p (a b)"), ccv_out[0])

            # KT[e,k]: keys 0..1023 (chunks 0,1) are projected on BOTH
            # cores of the pair -- window 0's scores need them before any
            # collective could land.  Keys 1024..2047 are only needed by
            # window 1 (~30us later), so each core projects just chunk 2+h
            # and a second small AllGather distributes that half.
            for c in range(2):
                for et in range(8):
                    ps = pproj.tile([P, 512], F32, tag="pp")
                    for dt in range(8):
                        nc.tensor.matmul(
                            ps[:], wk_sb[:, dt, P * et:P * (et + 1)],
                            xt_tiles[c][:, dt, :],
                            start=(dt == 0), stop=(dt == 7))
                    nc.scalar.copy(KT[:, et, 512 * c:512 * (c + 1)], ps[:])
            for et in range(8):
                ps = pproj.tile([P, 512], F32, tag="pp")
                for dt in range(8):
                    nc.tensor.matmul(
                        ps[:], wk_sb[:, dt, P * et:P * (et + 1)],
                        xk_sb[:, dt, :],
                        start=(dt == 0), stop=(dt == 7))
                nc.scalar.copy(kth[:, et, :], ps[:])

            cck_in = ccdram.tile([P, 8 * 512], BF16, tag="cck_in")
            cck_out = ccdram.tile([2, P, 8 * 512], BF16, tag="cck_out")
            nc.gpsimd.dma_start(cck_in[:],
                                kth[:].rearrange("p a b -> p (a b)"))
            nc.gpsimd.collective_compute(
                "AllGather", mybir.AluOpType.bypass,
                replica_groups=GROUPS,
                ins=[cck_in[:]], outs=[cck_out[:]])
            nc.gpsimd.dma_start(
                V4[:, 1].rearrange("p a b -> p (a b)"), ccv_out[1])
            for r in range(2):
                nc.gpsimd.dma_start(
                    KT[:, :, 1024 + 512 * r:1024 + 512 * (r + 1)],
                    cck_out[r].rearrange("p (a b) -> p a b", a=8))

            # QT[e,q]: stationary wq e-tile, moving xq half.  Last so the
            # K collective's flight time hides under it.
            for et in range(8):
                for qh in range(2):
                    ps = pproj.tile([P, 512], F32, tag="pp")
                    for dt in range(8):
                        nc.tensor.matmul(
                            ps[:], wq_sb[:, dt, P * et:P * (et + 1)],
                            xq_sb[:, qh, dt, :],
                            start=(dt == 0), stop=(dt == 7))
                    nc.scalar.copy(QT[:, et, 512 * qh:512 * (qh + 1)], ps[:])

        # ---------------- attention phase ----------------
        # Queries are owned as 128-row blocks {2m+h}; window w covers the
        # core's query tiles j=0..3 (global blocks 2(4w+j)+h).  The k range
        # shrinks in a 128-column staircase: at k-tile kt only query tiles
        # j >= jmin(kt) are still active, so the scores matmul runs at
        # N = 512-128*jmin.  Each query tile's last two k-tiles carry a
        # data-driven diag/zero (h=0) or ones/diag (h=1) mask, applied
        # in place on a [P,128] slice.  PV runs with expT tiles stationary,
        # producing out[q,e] directly (no PE transposes); the softmax
        # denominator is folded in via a DVE tensor_scalar multiply during
        # psum evacuation.
        with tc.tile_pool(name="ps_s", bufs=2, space="PSUM") as ps_s, \
             tc.tile_pool(name="ps_d", bufs=1, space="PSUM") as ps_d, \
             tc.tile_pool(name="ps_o", bufs=4, space="PSUM") as ps_o, \
             tc.tile_pool(name="expp", bufs=2) as expp, \
             tc.tile_pool(name="rawp", bufs=2) as rawp, \
             tc.tile_pool(name="osb", bufs=4) as osbp, \
             tc.tile_pool(name="dendram", bufs=1, space="DRAM") as dendramp, \
             tc.tile_pool(name="dsb", bufs=2) as dsbp:

            den_dram = dendramp.tile([2, 2 * QB], F32)

            for w in range(2):
                kmax = 8 * w + 8
                expbuf = expp.tile([P, 16, 512], BF16, tag="expbuf")
                pd_l = ps_d.tile([P, 512], F32, tag="pdl")
                pd_r = ps_d.tile([P, 512], F32, tag="pdr")

                def jmin(kt, w=w):
                    return max(0, (kt - 8 * w) // 2)

                # denominator matmuls for tile k; emitted one iteration late
                # (software pipeline) so PE never waits on the exp latency.
                # Query tiles {0,1} and {2,3} accumulate in separate psum
                # banks so the first pair's reciprocal is ready (and
                # readable without a bank hazard) before the window's
                # staircase finishes, unblocking the first PV evacuations.
                def emit_den(k, pd_l=pd_l, pd_r=pd_r, w=w, kmax=kmax,
                             expbuf=expbuf):
                    off = P * jmin(k, w)
                    if off < 256:
                        nc.tensor.matmul(pd_l[0:1, off:256], ones[:, 0:1],
                                         expbuf[:, k, off:256],
                                         start=(k == 0),
                                         stop=(k == 8 * w + 3))
                    roff = max(256, off)
                    nc.tensor.matmul(pd_r[0:1, roff:512], ones[:, 0:1],
                                     expbuf[:, k, roff:512],
                                     start=(k == 0), stop=(k == kmax - 1))

                # denominator -> [q,1] layout via DRAM roundtrip, one half
                # (query-tile pair) at a time: the first half's roundtrip is
                # emitted as soon as its accumulation stops
                dsb = dsbp.tile([1, 2, 256], F32, tag="den", name=f"dsb{w}")

                def emit_denrt(sh, w=w, dsb=dsb, pd_l=pd_l, pd_r=pd_r):
                    nc.vector.tensor_copy(dsb[:, sh],
                                          (pd_l if sh == 0 else
                                           pd_r)[0:1, 256 * sh:
                                                 256 * (sh + 1)])
                    nc.sync.dma_start(den_dram[w:w + 1, 256 * sh:
                                               256 * (sh + 1)],
                                      dsb[:, sh])
                    nc.sync.dma_start(
                        denT[:, 4 * w + 2 * sh:4 * w + 2 * sh + 2],
                        den_dram[w:w + 1, 256 * sh:256 * (sh + 1)]
                        .rearrange("o (c q) -> (o q) c", q=P))
                    nc.vector.reciprocal(
                        rinv[:, 4 * w + 2 * sh:4 * w + 2 * sh + 2],
                        denT[:, 4 * w + 2 * sh:4 * w + 2 * sh + 2])

                # scores + exp, staircase width
                for kt in range(kmax):
                    off = P * jmin(kt)
                    ps = ps_s.tile([P, 512], F32, tag="ps")
                    for et in range(8):
                        nc.tensor.matmul(
                            ps[:, off:512], KT[:, et, P * kt:P * (kt + 1)],
                            QT[:, et, 512 * w + off:512 * (w + 1)],
                            start=(et == 0), stop=(et == 7))
                    nc.scalar.activation(expbuf[:, kt, off:512],
                                         ps[:, off:512],
                                         mybir.ActivationFunctionType.Exp,
                                         scale=SCALE)
                    if kt >= 8 * w:
                        # mask the diagonal query tile of this k-tile
                        j, par = divmod(kt - 8 * w, 2)
                        nc.vector.tensor_tensor(
                            expbuf[:, kt, P * j:P * (j + 1)],
                            expbuf[:, kt, P * j:P * (j + 1)],
                            mask_sb[:, par, :],
                            mybir.AluOpType.mult)
                    if kt >= 1:
                        emit_den(kt - 1)
                    if kt == 8 * w + 4:
                        emit_denrt(0)
                emit_den(kmax - 1)
                emit_denrt(1)

                # PV per 128-query tile: expT stationary, V moving; psum
                # accumulates out[q, 1024e] across the tile's k range.
                # Tiles 0,1 evacuate UNSCALED to SBUF first (their rinv may
                # still be in the denominator DMA roundtrip; copying frees
                # the psum banks so tiles 2,3 never stall on the pool),
                # then scale+store once rinv lands.  Tiles 2,3 scale from
                # psum directly (their rinv is always ready by then).
                raws = []

                def emit_deferred(jd, raw, w=w):
                    ob = osbp.tile([P, D], BF16, tag="ob",
                                   name=f"obd{w}{jd}")
                    for eh in range(2):
                        nc.vector.tensor_scalar_mul(
                            ob[:, 512 * eh:512 * (eh + 1)],
                            raw[:, eh, :],
                            rinv[:, 4 * w + jd:4 * w + jd + 1])
                        nc.sync.dma_start(
                            out[512 * w + P * jd:512 * w + P * (jd + 1),
                                512 * eh:512 * (eh + 1)],
                            ob[:, 512 * eh:512 * (eh + 1)])

                for j in range(4):
                    kcnt = 8 * w + 2 * j + 2
                    # two single-bank psum tiles: the second e-half's
                    # matmuls must not serialize behind the DVE evacuation
                    # of the first (a shared tile creates that false WAR)
                    poa = ps_o.tile([P, 512], F32, tag="po",
                                    name=f"po{w}{j}a")
                    pob = ps_o.tile([P, 512], F32, tag="po",
                                    name=f"po{w}{j}b")
                    ob = (osbp.tile([P, D], BF16, tag="ob",
                                    name=f"ob{w}{j}")
                          if j >= 2 else None)
                    raw = (rawp.tile([P, 2, 512], F32, tag="raw",
                                     name=f"raw{w}{j}")
                           if j < 2 else None)
                    for eh, po in ((0, poa), (1, pob)):
                        for kt in range(kcnt):
                            # global 128-row tile kt came from pair rank
                            # kt%2's own row-tile kt//2
                            jv, r = divmod(kt, 2)
                            nc.tensor.matmul(
                                po[:],
                                expbuf[:, kt, P * j:P * (j + 1)],
                                V4[:, r, jv, 512 * eh:512 * (eh + 1)],
                                start=(kt == 0), stop=(kt == kcnt - 1))
                        if j >= 2:
                            nc.vector.tensor_scalar_mul(
                                ob[:, 512 * eh:512 * (eh + 1)], po[:],
                                rinv[:, 4 * w + j:4 * w + j + 1])
                            nc.sync.dma_start(
                                out[512 * w + P * j:512 * w + P * (j + 1),
                                    512 * eh:512 * (eh + 1)],
                                ob[:, 512 * eh:512 * (eh + 1)])
                        else:
                            nc.vector.tensor_copy(raw[:, eh], po[:])
                    if j < 2:
                        raws.append(raw)
                    elif j == 2:
                        # rinv for tiles 0,1 has landed by now: emit their
                        # deferred scale+store (their psum banks were
                        # already freed by the raw copies)
                        for jd, raw in enumerate(raws):
                            emit_deferred(jd, raw)

    nc.compile()
    return nc


_NC_CACHE = None


def _get_nc():
    global _NC_CACHE
    if _NC_CACHE is None:
        _NC_CACHE = _build_kernel()
    return _NC_CACHE


def _make_masks():
    # mask_sb[:, par, :] multiplies the diagonal 128-query tile of k-tile
    # kt with (kt - 8w) % 2 == par.  The owned block is B = 8w+2j+h; the
    # masked k-tiles are kt = B+1-h (par h's "even"/"odd" slots):
    #   h=0: par 0 -> kt == B (diagonal), par 1 -> kt == B+1 (all masked)
    #   h=1: par 0 -> kt == B-1 (all allowed), par 1 -> kt == B (diagonal)
    kk = np.arange(P)[:, None]
    qq = np.arange(P)[None, :]
    diag0 = (qq >= kk).astype(np.float32)
    m = {}
    for h in range(2):
        mt = np.zeros((P, 2, P), np.float32)
        if h == 0:
            mt[:, 0] = diag0
        else:
            mt[:, 0] = 1.0
            mt[:, 1] = diag0
        m[h] = mt.astype(ml_dtypes.bfloat16)
    return m


def _prep_inputs(x, Wq, Wk, Wv):
    bf16 = ml_dtypes.bfloat16

    def wfull(W):
        # [d_in, e] -> [p, dt, e]
        return np.ascontiguousarray(
            np.asarray(W, np.float32).reshape(8, P, D).transpose(1, 0, 2)
        ).astype(bf16)

    def whalves(W):
        # [d_in, e] -> [eh, p, dt, 512]
        wf = np.asarray(W, np.float32).reshape(8, P, 2, 512)
        return np.ascontiguousarray(wf.transpose(2, 1, 0, 3)).astype(bf16)

    wq4, wk4, wv4 = wfull(Wq), wfull(Wk), whalves(Wv)
    masks = _make_masks()
    in_maps = []
    for core in range(NCORES):
        b, h = divmod(core, 2)
        xb = np.asarray(x[b], np.float32)
        xt4 = np.ascontiguousarray(
            xb.reshape(4, 512, 8, P).transpose(0, 3, 2, 1)).astype(bf16)
        xk4 = np.ascontiguousarray(xt4[2 + h])
        order = np.concatenate(
            [np.arange(P * (2 * m + h), P * (2 * m + h) + P)
             for m in range(8)])
        xq = xb[order]
        xq4 = np.ascontiguousarray(
            xq.reshape(2, 512, 8, P).transpose(0, 3, 2, 1)).astype(bf16)
        in_maps.append({
            "xtd": np.ascontiguousarray(xt4[0:2]), "xkd": xk4, "xqd": xq4,
            "wqd": wq4, "wkd": wk4, "wvd": wv4,
            "maskT": masks[h],
        })
    return in_maps


def run(inputs, trace=False):
    nc = _get_nc()
    in_maps = _prep_inputs(inputs["x"], inputs["Wq"], inputs["Wk"],
                           inputs["Wv"])
    res = bass_utils.run_bass_kernel_spmd(
        nc, in_maps, core_ids=list(range(NCORES)), trace=trace)
    out = np.empty((B, S, D), np.float32)
    for core in range(NCORES):
        b, h = divmod(core, 2)
        oc = np.asarray(res.results[core]["out"]).astype(np.float32)
        for m in range(8):
            out[b, P * (2 * m + h):P * (2 * m + h) + P] = \
                oc[P * m:P * m + P]
    return out, res


def kernel(**inputs):
    out, _ = run(inputs, trace=False)
    return out
